# revision 2
# baseline (speedup 1.0000x reference)
# Trainium2 Bass kernel for nn_SSM: gated exponential-FIR depthwise conv + projections.
#
#   dynamic_scale = sigmoid(x @ Wd^T + bd)
#   u  = x * tanh(raw_beta)
#   y  = causal depthwise FIR conv (K=128) of u with kernel A[d]*r[d]^k (L2-normalized),
#        cross-correlated => lag-j tap is A*r^(127-j)
#   out = (y * dynamic_scale + x) @ Wo^T + bo
#
# Strategy: data-parallel over tokens (B*T = 16384 -> 2048/core, 8 cores), halo of 128
# past tokens per core. Everything on-chip runs in TIME-REVERSED transposed layout
# [channel, tau'] so the backward FIR becomes a forward first-order scan:
#     y'[tau'] = r*y'[tau'-1] + (u[tau'+127] - r^128 * u[tau'-1])      (hw tensor_tensor_scan)
# with per-channel scale (tanh(beta)*A) folded into the gating multiply.
# Matmuls run in fp32r (fp32 with 11-bit mantissa) at full PE rate.
import numpy as np
import concourse.bacc as bacc
import concourse.tile as tile
import concourse.mybir as mybir
from concourse import bass_utils

F32 = mybir.dt.float32
F32R = mybir.dt.float32r
AL = mybir.AluOpType
ACTF = mybir.ActivationFunctionType

B, T, D = 4, 4096, 1024
NCORE = 8
TC = (B * T) // NCORE          # tokens per core (2048)
CH = 256                       # time chunk
NCH = TC // CH                 # 8 chunks
KD = D // 128                  # 8 channel tiles
XW = TC + 129                  # x columns: col j = u_rev[j-1], col 0 is zero pad

_CACHE = {}


def _build():
    if "nc" in _CACHE:
        return _CACHE["nc"]
    nc = bacc.Bacc("TRN2", target_bir_lowering=False, debug=False, num_devices=NCORE)

    XT = nc.dram_tensor("xt", [D, XW], F32R, kind="ExternalInput")
    WDT = nc.dram_tensor("wdt", [D, D], F32R, kind="ExternalInput")
    WOT = nc.dram_tensor("wot", [D, D], F32R, kind="ExternalInput")
    # per-channel vectors packed [128, KD]: vec[p, k] = v[k*128 + p]
    RV = nc.dram_tensor("rv", [128, KD], F32, kind="ExternalInput")      # r
    NR128 = nc.dram_tensor("nr128", [128, KD], F32, kind="ExternalInput")  # -r^128
    SCV = nc.dram_tensor("scv", [128, KD], F32, kind="ExternalInput")    # tanh(beta)*A
    BDV = nc.dram_tensor("bdv", [128, KD], F32, kind="ExternalInput")    # bd
    BOV = nc.dram_tensor("bov", [128, KD], F32, kind="ExternalInput")    # bo
    OUT = nc.dram_tensor("out", [D, TC], F32, kind="ExternalOutput")

    with tile.TileContext(nc) as tc:
        with (
            tc.tile_pool(name="wgt", bufs=1) as wgt,
            tc.tile_pool(name="cst", bufs=1) as cst,
            tc.tile_pool(name="xp", bufs=2) as xp,
            tc.tile_pool(name="d1p", bufs=1) as d1p,
            tc.tile_pool(name="yp", bufs=2) as yp,
            tc.tile_pool(name="wp", bufs=1) as wp,
            tc.tile_pool(name="vp", bufs=2) as vp,
            tc.tile_pool(name="gp", bufs=2) as gp,
            tc.tile_pool(name="op", bufs=2) as op,
            tc.tile_pool(name="psg", bufs=4, space="PSUM") as psg,
            tc.tile_pool(name="pso", bufs=4, space="PSUM") as pso,
        ):
            # ---- static loads
            wd_t, wo_t = [], []
            for k in range(KD):
                wd = wgt.tile([128, D], F32R, tag=f"wd{k}")
                nc.sync.dma_start(wd[:], WDT[k * 128:(k + 1) * 128, :])
                wd_t.append(wd)
            for k in range(KD):
                wo = wgt.tile([128, D], F32R, tag=f"wo{k}")
                nc.sync.dma_start(wo[:], WOT[k * 128:(k + 1) * 128, :])
                wo_t.append(wo)
            r_t = cst.tile([128, KD], F32, tag="r")
            nc.sync.dma_start(r_t[:], RV[:])
            nr_t = cst.tile([128, KD], F32, tag="nr")
            nc.sync.dma_start(nr_t[:], NR128[:])
            sc_t = cst.tile([128, KD], F32, tag="sc")
            nc.sync.dma_start(sc_t[:], SCV[:])
            bd_t = cst.tile([128, KD], F32, tag="bd")
            nc.sync.dma_start(bd_t[:], BDV[:])
            bo_t = cst.tile([128, KD], F32, tag="bo")
            nc.sync.dma_start(bo_t[:], BOV[:])

            # ---- x chunk 0 + boot scan (y'[-1] = scan of u_rev[0..126])
            xt = {}
            for k in range(KD):
                t = xp.tile([128, 385], F32R, tag=f"x{k}")
                nc.sync.dma_start(t[:], XT[k * 128:(k + 1) * 128, 0:385])
                xt[(0, k)] = t
            ylast = []
            for k in range(KD):
                zp = cst.tile([128, 127], F32, tag=f"zp{k}")
                nc.vector.tensor_tensor_scan(
                    zp[:],
                    r_t[:, k:k + 1].broadcast_to([128, 127]),
                    xt[(0, k)][:, 1:128].bitcast(F32),
                    0.0, AL.mult, AL.add,
                )
                ylast.append(zp[:, 126:127])

            # ---- main chunk loop
            for c in range(NCH):
                # prefetch next x chunk
                if c + 1 < NCH:
                    for k in range(KD):
                        t = xp.tile([128, 385], F32R, tag=f"x{k}")
                        nc.sync.dma_start(
                            t[:], XT[k * 128:(k + 1) * 128, (c + 1) * CH:(c + 1) * CH + 385])
                        xt[(c + 1, k)] = t

                # gate matmuls + sigmoid (PE/ACT; independent of scan)
                g_t = []
                for j in range(KD):
                    pg = psg.tile([128, CH], F32, tag="pg")
                    for k in range(KD):
                        nc.tensor.matmul(
                            pg[:],
                            wd_t[k][:, j * 128:(j + 1) * 128],
                            xt[(c, k)][:, 1:257],
                            start=(k == 0), stop=(k == KD - 1),
                        )
                    g = gp.tile([128, CH], F32, tag=f"g{j}")
                    nc.scalar.activation(g[:], pg[:], ACTF.Sigmoid, bias=bd_t[:, j:j + 1])
                    g_t.append(g)

                # scan chain + gating + residual (DVE)
                v_t = []
                for k in range(KD):
                    xa = xt[(c, k)]
                    d1 = d1p.tile([128, CH], F32, tag=f"d{k}")
                    nc.vector.scalar_tensor_tensor(
                        d1[:], xa[:, 0:256].bitcast(F32), nr_t[:, k:k + 1],
                        xa[:, 128:384].bitcast(F32), AL.mult, AL.add)
                    y = yp.tile([128, CH], F32, tag=f"y{k}")
                    nc.vector.tensor_tensor_scan(
                        y[:], r_t[:, k:k + 1].broadcast_to([128, CH]), d1[:],
                        ylast[k], AL.mult, AL.add)
                    ylast[k] = y[:, CH - 1:CH]
                    w = wp.tile([128, CH], F32, tag=f"w{k}")
                    nc.vector.scalar_tensor_tensor(
                        w[:], y[:], sc_t[:, k:k + 1], g_t[k][:], AL.mult, AL.mult)
                    v = vp.tile([128, CH], F32R, tag=f"v{k}")
                    nc.vector.tensor_tensor(
                        v[:], w[:], xa[:, 1:257].bitcast(F32), AL.add)
                    v_t.append(v)

                # output projection + bias + store
                for j in range(KD):
                    po = pso.tile([128, CH], F32, tag="po")
                    for k in range(KD):
                        nc.tensor.matmul(
                            po[:],
                            wo_t[k][:, j * 128:(j + 1) * 128],
                            v_t[k][:],
                            start=(k == 0), stop=(k == KD - 1),
                        )
                    o = op.tile([128, CH], F32, tag=f"o{j}")
                    nc.scalar.activation(o[:], po[:], ACTF.Identity, bias=bo_t[:, j:j + 1])
                    nc.sync.dma_start(
                        OUT[j * 128:(j + 1) * 128, c * CH:(c + 1) * CH], o[:])

    nc.compile()
    _CACHE["nc"] = nc
    return nc


def _pack_vec(v):
    # [D] -> [128, KD] with vec[p, k] = v[k*128+p]
    return np.ascontiguousarray(v.astype(np.float32).reshape(KD, 128).T)


def kernel(x, raw_gamma, raw_beta, C, Wd, bd, Wo, bo):
    x = np.asarray(x)
    in_dtype = x.dtype
    # host precompute of per-channel scalars (f64 -> f32)
    rg = np.asarray(raw_gamma, np.float64)
    rb = np.asarray(raw_beta, np.float64)
    C64 = np.asarray(C, np.float64)
    gamma = np.log1p(np.exp(-np.abs(rg))) + np.maximum(rg, 0.0) + 1e-4  # softplus
    r = np.exp(-gamma)
    decay = np.exp(-np.arange(128)[None, :] * gamma[:, None])
    ker = C64[:, None] * decay
    norm = np.sqrt((ker * ker).sum(-1))
    A = C64 / (norm + 1e-6)
    scale = np.tanh(rb) * A
    r128 = r ** 128

    nc = _build()

    xf = np.asarray(x, np.float32).reshape(B * T, D)
    wdt = np.ascontiguousarray(np.asarray(Wd, np.float32).T)
    wot = np.ascontiguousarray(np.asarray(Wo, np.float32).T)
    rv = _pack_vec(r)
    nr128 = _pack_vec(-r128)
    scv = _pack_vec(scale)
    bdv = _pack_vec(np.asarray(bd, np.float64))
    bov = _pack_vec(np.asarray(bo, np.float64))

    in_maps = []
    for c in range(NCORE):
        t0 = c * TC
        b = t0 // T
        lo = max(b * T, t0 - 128)
        # x_rev: col 0 = zero, cols 1..TC = chunk reversed, then halo reversed, zero pad
        xr = np.zeros((XW, D), np.float32)
        xr[1:TC + 1] = xf[t0:t0 + TC][::-1]
        nh = t0 - lo
        if nh:
            xr[TC + 1:TC + 1 + nh] = xf[lo:t0][::-1]
        in_maps.append({
            "xt": np.ascontiguousarray(xr.T),
            "wdt": wdt, "wot": wot,
            "rv": rv, "nr128": nr128, "scv": scv, "bdv": bdv, "bov": bov,
        })

    _CACHE["last_in_maps"] = in_maps
    res = bass_utils.run_bass_kernel_spmd(nc, in_maps, core_ids=list(range(NCORE)))
    _CACHE["last_result"] = res

    out = np.empty((B * T, D), np.float32)
    for c in range(NCORE):
        o = res.results[c]["out"]                      # [D, TC] reversed time
        out[c * TC:(c + 1) * TC] = o[:, ::-1].T
    return out.reshape(B, T, D).astype(in_dtype, copy=False)


# revision 7
# speedup vs baseline: 1.0021x; 1.0021x over previous
# Trainium2 Bass kernel for nn_SSM: gated exponential-FIR depthwise conv + projections.
#
#   dynamic_scale = sigmoid(x @ Wd^T + bd)
#   u  = x * tanh(raw_beta)
#   y  = causal depthwise FIR conv (K=128) of u with kernel A[d]*r[d]^k (L2-normalized),
#        cross-correlated => lag-j tap is A*r^(127-j)
#   out = (y * dynamic_scale + x) @ Wo^T + bo
#
# Strategy: data-parallel over tokens (B*T = 16384 -> 2048/core, 8 cores), halo of 128
# past tokens per core. Everything on-chip runs in TIME-REVERSED transposed layout
# [channel, tau'] so the backward FIR becomes a forward first-order scan:
#     y'[tau'] = r*y'[tau'-1] + (u[tau'+127] - r^128 * u[tau'-1])      (hw tensor_tensor_scan)
# with per-channel scale (tanh(beta)*A) folded into the gating multiply.
# Matmuls run in fp32r (fp32 with 11-bit mantissa) at full PE rate.
import numpy as np
import concourse.bacc as bacc
import concourse.tile as tile
import concourse.mybir as mybir
from concourse import bass_utils

F32 = mybir.dt.float32
F32R = mybir.dt.float32r
AL = mybir.AluOpType
ACTF = mybir.ActivationFunctionType

B, T, D = 4, 4096, 1024
NCORE = 8
TC = (B * T) // NCORE          # tokens per core (2048)
CH = 256                       # time chunk
NCH = TC // CH                 # 8 chunks
KD = D // 128                  # 8 channel tiles
XW = TC + 129                  # x columns: col j = u_rev[j-1], col 0 is zero pad

_CACHE = {}


def _build():
    if "nc" in _CACHE:
        return _CACHE["nc"]
    nc = bacc.Bacc("TRN2", target_bir_lowering=False, debug=False, num_devices=NCORE)

    XT = nc.dram_tensor("xt", [D, XW], F32R, kind="ExternalInput")
    WDT = nc.dram_tensor("wdt", [D, D], F32R, kind="ExternalInput")
    WOT = nc.dram_tensor("wot", [D, D], F32R, kind="ExternalInput")
    # per-channel vectors packed [128, 5*KD]: columns = KD each of
    # r, -r^128, tanh(beta)*A, bd, bo;  vec[p, k] = v[k*128 + p]
    CV = nc.dram_tensor("cv", [128, 5 * KD], F32, kind="ExternalInput")
    OUT = nc.dram_tensor("out", [D, TC], F32, kind="ExternalOutput")

    with tile.TileContext(nc) as tc:
        with (
            tc.tile_pool(name="wgt", bufs=1) as wgt,
            tc.tile_pool(name="cst", bufs=1) as cst,
            tc.tile_pool(name="xp", bufs=2) as xp,
            tc.tile_pool(name="d1p", bufs=1) as d1p,
            tc.tile_pool(name="yp", bufs=2) as yp,
            tc.tile_pool(name="wp", bufs=1) as wp,
            tc.tile_pool(name="vp", bufs=2) as vp,
            tc.tile_pool(name="gp", bufs=2) as gp,
            tc.tile_pool(name="op", bufs=2) as op,
            tc.tile_pool(name="psg", bufs=4, space="PSUM") as psg,
            tc.tile_pool(name="pso", bufs=4, space="PSUM") as pso,
        ):
            # ---- consts + x chunk 0 first (unblocks DVE boot + first chunk fast)
            cv_t = cst.tile([128, 5 * KD], F32, tag="cv")
            nc.sync.dma_start(cv_t[:], CV[:])
            r_t = cv_t[:, 0 * KD:1 * KD]
            nr_t = cv_t[:, 1 * KD:2 * KD]
            sc_t = cv_t[:, 2 * KD:3 * KD]
            bd_t = cv_t[:, 3 * KD:4 * KD]
            bo_t = cv_t[:, 4 * KD:5 * KD]

            xt = {}
            for k in range(KD):
                t = xp.tile([128, 385], F32R, tag=f"x{k}")
                nc.sync.dma_start(t[:], XT[k * 128:(k + 1) * 128, 0:385])
                xt[(0, k)] = t

            # boot scan (y'[-1] = scan of u_rev[0..126])
            ylast = []
            for k in range(KD):
                zp = cst.tile([128, 127], F32, tag=f"zp{k}")
                nc.vector.tensor_tensor_scan(
                    zp[:],
                    r_t[:, k:k + 1].broadcast_to([128, 127]),
                    xt[(0, k)][:, 1:128].bitcast(F32),
                    0.0, AL.mult, AL.add,
                )
                ylast.append(zp[:, 126:127])

            # ---- weights: Wd (needed first), x chunk 1 prefetch, then Wo
            wd_t, wo_t = [], []
            for k in range(KD):
                wd = wgt.tile([128, D], F32R, tag=f"wd{k}")
                nc.sync.dma_start(wd[:], WDT[k * 128:(k + 1) * 128, :])
                wd_t.append(wd)
            for k in range(KD):
                t = xp.tile([128, 385], F32R, tag=f"x{k}")
                nc.sync.dma_start(t[:], XT[k * 128:(k + 1) * 128, CH:CH + 385])
                xt[(1, k)] = t
            for k in range(KD):
                wo = wgt.tile([128, D], F32R, tag=f"wo{k}")
                nc.sync.dma_start(wo[:], WOT[k * 128:(k + 1) * 128, :])
                wo_t.append(wo)

            # ---- main chunk loop
            for c in range(NCH):
                # prefetch next x chunk (chunk 1 already prefetched above)
                if 1 <= c and c + 1 < NCH:
                    for k in range(KD):
                        t = xp.tile([128, 385], F32R, tag=f"x{k}")
                        nc.sync.dma_start(
                            t[:], XT[k * 128:(k + 1) * 128, (c + 1) * CH:(c + 1) * CH + 385])
                        xt[(c + 1, k)] = t

                # gate matmuls + sigmoid (PE/ACT; independent of scan)
                g_t = []
                for j in range(KD):
                    pg = psg.tile([128, CH], F32, tag="pg")
                    for k in range(KD):
                        nc.tensor.matmul(
                            pg[:],
                            wd_t[k][:, j * 128:(j + 1) * 128],
                            xt[(c, k)][:, 1:257],
                            start=(k == 0), stop=(k == KD - 1),
                        )
                    g = gp.tile([128, CH], F32, tag=f"g{j}")
                    nc.scalar.activation(g[:], pg[:], ACTF.Sigmoid, bias=bd_t[:, j:j + 1])
                    g_t.append(g)

                # scan chain + gating + residual (DVE)
                v_t = []
                for k in range(KD):
                    xa = xt[(c, k)]
                    d1 = d1p.tile([128, CH], F32, tag=f"d{k}")
                    nc.vector.scalar_tensor_tensor(
                        d1[:], xa[:, 0:256].bitcast(F32), nr_t[:, k:k + 1],
                        xa[:, 128:384].bitcast(F32), AL.mult, AL.add)
                    y = yp.tile([128, CH], F32, tag=f"y{k}")
                    nc.vector.tensor_tensor_scan(
                        y[:], r_t[:, k:k + 1].broadcast_to([128, CH]), d1[:],
                        ylast[k], AL.mult, AL.add)
                    ylast[k] = y[:, CH - 1:CH]
                    w = wp.tile([128, CH], F32, tag=f"w{k}")
                    nc.vector.scalar_tensor_tensor(
                        w[:], y[:], sc_t[:, k:k + 1], g_t[k][:], AL.mult, AL.mult)
                    v = vp.tile([128, CH], F32R, tag=f"v{k}")
                    nc.vector.tensor_tensor(
                        v[:], w[:], xa[:, 1:257].bitcast(F32), AL.add)
                    v_t.append(v)

                # output projection + bias + store
                for j in range(KD):
                    po = pso.tile([128, CH], F32, tag="po")
                    for k in range(KD):
                        nc.tensor.matmul(
                            po[:],
                            wo_t[k][:, j * 128:(j + 1) * 128],
                            v_t[k][:],
                            start=(k == 0), stop=(k == KD - 1),
                        )
                    o = op.tile([128, CH], F32, tag=f"o{j}")
                    nc.scalar.activation(o[:], po[:], ACTF.Identity, bias=bo_t[:, j:j + 1])
                    nc.sync.dma_start(
                        OUT[j * 128:(j + 1) * 128, c * CH:(c + 1) * CH], o[:])

    nc.compile()
    _CACHE["nc"] = nc
    return nc


def _pack_vec(v):
    # [D] -> [128, KD] with vec[p, k] = v[k*128+p]
    return np.ascontiguousarray(v.astype(np.float32).reshape(KD, 128).T)


def kernel(x, raw_gamma, raw_beta, C, Wd, bd, Wo, bo):
    x = np.asarray(x)
    in_dtype = x.dtype
    # host precompute of per-channel scalars (f64 -> f32)
    rg = np.asarray(raw_gamma, np.float64)
    rb = np.asarray(raw_beta, np.float64)
    C64 = np.asarray(C, np.float64)
    gamma = np.log1p(np.exp(-np.abs(rg))) + np.maximum(rg, 0.0) + 1e-4  # softplus
    r = np.exp(-gamma)
    decay = np.exp(-np.arange(128)[None, :] * gamma[:, None])
    ker = C64[:, None] * decay
    norm = np.sqrt((ker * ker).sum(-1))
    A = C64 / (norm + 1e-6)
    scale = np.tanh(rb) * A
    r128 = r ** 128

    nc = _build()

    xf = np.asarray(x, np.float32).reshape(B * T, D)
    wdt = np.ascontiguousarray(np.asarray(Wd, np.float32).T)
    wot = np.ascontiguousarray(np.asarray(Wo, np.float32).T)
    cv = np.concatenate([
        _pack_vec(r), _pack_vec(-r128), _pack_vec(scale),
        _pack_vec(np.asarray(bd, np.float64)), _pack_vec(np.asarray(bo, np.float64)),
    ], axis=1)

    in_maps = []
    for c in range(NCORE):
        t0 = c * TC
        b = t0 // T
        lo = max(b * T, t0 - 128)
        # x_rev: col 0 = zero, cols 1..TC = chunk reversed, then halo reversed, zero pad
        xr = np.zeros((XW, D), np.float32)
        xr[1:TC + 1] = xf[t0:t0 + TC][::-1]
        nh = t0 - lo
        if nh:
            xr[TC + 1:TC + 1 + nh] = xf[lo:t0][::-1]
        in_maps.append({
            "xt": np.ascontiguousarray(xr.T),
            "wdt": wdt, "wot": wot, "cv": cv,
        })

    _CACHE["last_in_maps"] = in_maps
    res = bass_utils.run_bass_kernel_spmd(nc, in_maps, core_ids=list(range(NCORE)))
    _CACHE["last_result"] = res

    out = np.empty((B * T, D), np.float32)
    for c in range(NCORE):
        o = res.results[c]["out"]                      # [D, TC] reversed time
        out[c * TC:(c + 1) * TC] = o[:, ::-1].T
    return out.reshape(B, T, D).astype(in_dtype, copy=False)


# revision 10
# speedup vs baseline: 1.0383x; 1.0361x over previous
# Trainium2 Bass kernel for nn_SSM: gated exponential-FIR depthwise conv + projections.
#
#   dynamic_scale = sigmoid(x @ Wd^T + bd)
#   u  = x * tanh(raw_beta)
#   y  = causal depthwise FIR conv (K=128) of u, kernel A[d]*r[d]^k (L2-normalized),
#        cross-correlated => lag-j tap is A*r^(127-j)
#   out = (y * dynamic_scale + x) @ Wo^T + bo
#
# Data-parallel over tokens (B*T = 16384 -> 2048/core, 8 cores), halo of 127 past
# tokens per core. On-chip everything runs in TIME-REVERSED transposed layout
# [channel, tau'] so the backward FIR becomes a forward first-order scan:
#     y'[tau'] = r*y'[tau'-1] + (u[tau'+127] - r^128 * u[tau'-1])   (hw tensor_tensor_scan)
# with the per-channel scale (tanh(beta)*A) folded into the gating multiply.
# Matmuls run in fp32r (fp32 with 11-bit mantissa) at full PE rate, N=512.
import numpy as np
import concourse.bacc as bacc
import concourse.tile as tile
import concourse.mybir as mybir
from concourse import bass_utils

F32 = mybir.dt.float32
F32R = mybir.dt.float32r
AL = mybir.AluOpType
ACTF = mybir.ActivationFunctionType

B, T, D = 4, 4096, 1024
NCORE = 8
TC = (B * T) // NCORE          # tokens per core (2048)
CH = 512                       # time chunk
NCH = TC // CH                 # 4 chunks
KD = D // 128                  # 8 channel tiles
XW = TC + 129                  # x columns: col j = u_rev[j-1], col 0 is zero pad

_CACHE = {}


def _build():
    if "nc" in _CACHE:
        return _CACHE["nc"]
    nc = bacc.Bacc("TRN2", target_bir_lowering=False, debug=False, num_devices=NCORE)

    XT = nc.dram_tensor("xt", [D, XW], F32R, kind="ExternalInput")
    WDT = nc.dram_tensor("wdt", [D, D], F32R, kind="ExternalInput")
    WOT = nc.dram_tensor("wot", [D, D], F32R, kind="ExternalInput")
    # per-channel vectors packed [128, 5*KD]: KD cols each of
    # r, -r^128, tanh(beta)*A, bd, bo;  vec[p, k] = v[k*128 + p]
    CV = nc.dram_tensor("cv", [128, 5 * KD], F32, kind="ExternalInput")
    OUT = nc.dram_tensor("out", [D, TC], F32, kind="ExternalOutput")

    with tile.TileContext(nc) as tc:
        with (
            tc.tile_pool(name="wgt", bufs=1) as wgt,
            tc.tile_pool(name="cst", bufs=1) as cst,
            tc.tile_pool(name="xp", bufs=2) as xp,
            tc.tile_pool(name="d1p", bufs=2) as d1p,
            tc.tile_pool(name="yp", bufs=2) as yp,
            tc.tile_pool(name="wp", bufs=2) as wp,
            tc.tile_pool(name="vp", bufs=1) as vp,
            tc.tile_pool(name="gp", bufs=1) as gp,
            tc.tile_pool(name="op", bufs=1) as op,
            tc.tile_pool(name="psg", bufs=3, space="PSUM") as psg,
            tc.tile_pool(name="pso", bufs=3, space="PSUM") as pso,
        ):
            # ---- consts + x chunk 0 first (unblocks DVE boot + first chunk fast)
            cv_t = cst.tile([128, 5 * KD], F32, tag="cv")
            nc.sync.dma_start(cv_t[:], CV[:])
            r_t = cv_t[:, 0 * KD:1 * KD]
            nr_t = cv_t[:, 1 * KD:2 * KD]
            sc_t = cv_t[:, 2 * KD:3 * KD]
            bd_t = cv_t[:, 3 * KD:4 * KD]
            bo_t = cv_t[:, 4 * KD:5 * KD]

            xt = {}
            for k in range(KD):
                t = xp.tile([128, 640], F32R, tag=f"x{k}")
                nc.sync.dma_start(t[:], XT[k * 128:(k + 1) * 128, 0:640])
                xt[(0, k)] = t

            # boot scan (y'[-1] = scan of u_rev[0..126])
            ylast = []
            for k in range(KD):
                zp = cst.tile([128, 127], F32, tag=f"zp{k}")
                nc.vector.tensor_tensor_scan(
                    zp[:],
                    r_t[:, k:k + 1].broadcast_to([128, 127]),
                    xt[(0, k)][:, 1:128].bitcast(F32),
                    0.0, AL.mult, AL.add,
                )
                ylast.append(zp[:, 126:127])

            # ---- weights: Wd (needed first), x chunk 1 prefetch, then Wo
            wd_t, wo_t = [], []
            for k in range(KD):
                wd = wgt.tile([128, D], F32R, tag=f"wd{k}")
                nc.sync.dma_start(wd[:], WDT[k * 128:(k + 1) * 128, :])
                wd_t.append(wd)
            for k in range(KD):
                t = xp.tile([128, 640], F32R, tag=f"x{k}")
                nc.sync.dma_start(t[:], XT[k * 128:(k + 1) * 128, CH:CH + 640])
                xt[(1, k)] = t
            for k in range(KD):
                wo = wgt.tile([128, D], F32R, tag=f"wo{k}")
                nc.sync.dma_start(wo[:], WOT[k * 128:(k + 1) * 128, :])
                wo_t.append(wo)

            # ---- main chunk loop
            for c in range(NCH):
                if 1 <= c and c + 1 < NCH:
                    for k in range(KD):
                        t = xp.tile([128, 640], F32R, tag=f"x{k}")
                        nc.sync.dma_start(
                            t[:], XT[k * 128:(k + 1) * 128, (c + 1) * CH:(c + 1) * CH + 640])
                        xt[(c + 1, k)] = t

                # gate matmuls + sigmoid (PE/ACT; independent of scan chain)
                g_t = []
                for j in range(KD):
                    pg = psg.tile([128, CH], F32, tag="pg")
                    for k in range(KD):
                        nc.tensor.matmul(
                            pg[:],
                            wd_t[k][:, j * 128:(j + 1) * 128],
                            xt[(c, k)][:, 1:513],
                            start=(k == 0), stop=(k == KD - 1),
                        )
                    g = gp.tile([128, CH], F32, tag=f"g{j}")
                    nc.scalar.activation(g[:], pg[:], ACTF.Sigmoid, bias=bd_t[:, j:j + 1])
                    g_t.append(g)

                # scan chain + gating + residual (DVE)
                v_t = []
                for k in range(KD):
                    xa = xt[(c, k)]
                    d1 = d1p.tile([128, CH], F32, tag="d")
                    nc.vector.scalar_tensor_tensor(
                        d1[:], xa[:, 0:512].bitcast(F32), nr_t[:, k:k + 1],
                        xa[:, 128:640].bitcast(F32), AL.mult, AL.add)
                    y = yp.tile([128, CH], F32, tag=f"y{k}")
                    nc.vector.tensor_tensor_scan(
                        y[:], r_t[:, k:k + 1].broadcast_to([128, CH]), d1[:],
                        ylast[k], AL.mult, AL.add)
                    ylast[k] = y[:, CH - 1:CH]
                    w = wp.tile([128, CH], F32, tag="w")
                    nc.vector.scalar_tensor_tensor(
                        w[:], y[:], sc_t[:, k:k + 1], g_t[k][:], AL.mult, AL.mult)
                    v = vp.tile([128, CH], F32R, tag=f"v{k}")
                    nc.vector.tensor_tensor(
                        v[:], w[:], xa[:, 1:513].bitcast(F32), AL.add)
                    v_t.append(v)

                # output projection + bias + store
                for j in range(KD):
                    po = pso.tile([128, CH], F32, tag="po")
                    for k in range(KD):
                        nc.tensor.matmul(
                            po[:],
                            wo_t[k][:, j * 128:(j + 1) * 128],
                            v_t[k][:],
                            start=(k == 0), stop=(k == KD - 1),
                        )
                    o = op.tile([128, CH], F32, tag=f"o{j}")
                    nc.scalar.activation(o[:], po[:], ACTF.Identity, bias=bo_t[:, j:j + 1])
                    nc.sync.dma_start(
                        OUT[j * 128:(j + 1) * 128, c * CH:(c + 1) * CH], o[:])

    nc.compile()
    _CACHE["nc"] = nc
    return nc


def _pack_vec(v):
    # [D] -> [128, KD] with vec[p, k] = v[k*128+p]
    return np.ascontiguousarray(v.astype(np.float32).reshape(KD, 128).T)


def kernel(x, raw_gamma, raw_beta, C, Wd, bd, Wo, bo):
    x = np.asarray(x)
    in_dtype = x.dtype
    # host precompute of per-channel scalars (f64 -> f32)
    rg = np.asarray(raw_gamma, np.float64)
    rb = np.asarray(raw_beta, np.float64)
    C64 = np.asarray(C, np.float64)
    gamma = np.log1p(np.exp(-np.abs(rg))) + np.maximum(rg, 0.0) + 1e-4  # softplus
    r = np.exp(-gamma)
    decay = np.exp(-np.arange(128)[None, :] * gamma[:, None])
    ker = C64[:, None] * decay
    norm = np.sqrt((ker * ker).sum(-1))
    A = C64 / (norm + 1e-6)
    scale = np.tanh(rb) * A
    r128 = r ** 128

    nc = _build()

    xf = np.asarray(x, np.float32).reshape(B * T, D)
    wdt = np.ascontiguousarray(np.asarray(Wd, np.float32).T)
    wot = np.ascontiguousarray(np.asarray(Wo, np.float32).T)
    cv = np.concatenate([
        _pack_vec(r), _pack_vec(-r128), _pack_vec(scale),
        _pack_vec(np.asarray(bd, np.float64)), _pack_vec(np.asarray(bo, np.float64)),
    ], axis=1)

    in_maps = []
    for c in range(NCORE):
        t0 = c * TC
        b = t0 // T
        lo = max(b * T, t0 - 128)
        # x_rev: col 0 = zero, cols 1..TC = chunk reversed, then halo reversed, zero pad
        xr = np.zeros((XW, D), np.float32)
        xr[1:TC + 1] = xf[t0:t0 + TC][::-1]
        nh = t0 - lo
        if nh:
            xr[TC + 1:TC + 1 + nh] = xf[lo:t0][::-1]
        in_maps.append({
            "xt": np.ascontiguousarray(xr.T),
            "wdt": wdt, "wot": wot, "cv": cv,
        })

    _CACHE["last_in_maps"] = in_maps
    res = bass_utils.run_bass_kernel_spmd(nc, in_maps, core_ids=list(range(NCORE)))
    _CACHE["last_result"] = res

    out = np.empty((B * T, D), np.float32)
    for c in range(NCORE):
        o = res.results[c]["out"]                      # [D, TC] reversed time
        out[c * TC:(c + 1) * TC] = o[:, ::-1].T
    return out.reshape(B, T, D).astype(in_dtype, copy=False)


# revision 13
# speedup vs baseline: 1.0502x; 1.0115x over previous
# Trainium2 Bass kernel for nn_SSM: gated exponential-FIR depthwise conv + projections.
#
#   dynamic_scale = sigmoid(x @ Wd^T + bd)
#   u  = x * tanh(raw_beta)
#   y  = causal depthwise FIR conv (K=128) of u, kernel A[d]*r[d]^k (L2-normalized),
#        cross-correlated => lag-j tap is A*r^(127-j)
#   out = (y * dynamic_scale + x) @ Wo^T + bo
#
# Data-parallel over tokens (B*T = 16384 -> 2048/core, 8 cores), halo of 127 past
# tokens per core. On-chip everything runs in TIME-REVERSED transposed layout
# [channel, tau'] so the backward FIR becomes a forward first-order scan:
#     y'[tau'] = r*y'[tau'-1] + (u[tau'+127] - r^128 * u[tau'-1])   (hw tensor_tensor_scan)
# with the per-channel scale (tanh(beta)*A) folded into the gating multiply.
# Matmuls run in fp32r (fp32 with 11-bit mantissa) at full PE rate, N=512.
import numpy as np
import concourse.bacc as bacc
import concourse.tile as tile
import concourse.mybir as mybir
from concourse import bass_utils

F32 = mybir.dt.float32
F32R = mybir.dt.float32r
AL = mybir.AluOpType
ACTF = mybir.ActivationFunctionType

B, T, D = 4, 4096, 1024
NCORE = 8
TC = (B * T) // NCORE          # tokens per core (2048)
CH = 512                       # time chunk
NCH = TC // CH                 # 4 chunks
KD = D // 128                  # 8 channel tiles
XW = TC + 129                  # x columns: col j = u_rev[j-1], col 0 is zero pad

_CACHE = {}


def _build():
    if "nc" in _CACHE:
        return _CACHE["nc"]
    nc = bacc.Bacc("TRN2", target_bir_lowering=False, debug=False, num_devices=NCORE)

    XT = nc.dram_tensor("xt", [D, XW], F32R, kind="ExternalInput")
    WDT = nc.dram_tensor("wdt", [D, D], F32R, kind="ExternalInput")
    WOT = nc.dram_tensor("wot", [D, D], F32R, kind="ExternalInput")
    # per-channel vectors packed [128, 5*KD]: KD cols each of
    # r, -r^128, tanh(beta)*A, bd, bo;  vec[p, k] = v[k*128 + p]
    CV = nc.dram_tensor("cv", [128, 5 * KD], F32, kind="ExternalInput")
    OUT = nc.dram_tensor("out", [D, TC], F32, kind="ExternalOutput")

    with tile.TileContext(nc) as tc:
        with (
            tc.tile_pool(name="wgt", bufs=1) as wgt,
            tc.tile_pool(name="cst", bufs=1) as cst,
            tc.tile_pool(name="xp", bufs=2) as xp,
            tc.tile_pool(name="d1p", bufs=2) as d1p,
            tc.tile_pool(name="yp", bufs=2) as yp,
            tc.tile_pool(name="wp", bufs=2) as wp,
            tc.tile_pool(name="vp", bufs=1) as vp,
            tc.tile_pool(name="gp", bufs=1) as gp,
            tc.tile_pool(name="op", bufs=1) as op,
        ):
            # ---- consts + x chunk 0 first (unblocks DVE boot + first chunk fast)
            cv_t = cst.tile([128, 5 * KD], F32, tag="cv")
            nc.sync.dma_start(cv_t[:], CV[:])
            r_t = cv_t[:, 0 * KD:1 * KD]
            nr_t = cv_t[:, 1 * KD:2 * KD]
            sc_t = cv_t[:, 2 * KD:3 * KD]
            bd_t = cv_t[:, 3 * KD:4 * KD]
            bo_t = cv_t[:, 4 * KD:5 * KD]

            # ---- interleaved x0/Wd DMA pairs + boot scans, so the first gate
            # matmuls can start as soon as x0[0]+wd[0] land.
            xt = {}
            wd_t, wo_t = [], []
            ylast = [None] * KD
            for k in range(KD):
                t = xp.tile([128, 640], F32R, tag=f"x{k}")
                nc.sync.dma_start(t[:], XT[k * 128:(k + 1) * 128, 0:640])
                xt[(0, k)] = t
                wd = wgt.tile([128, D], F32R, tag=f"wd{k}")
                nc.sync.dma_start(wd[:], WDT[k * 128:(k + 1) * 128, :])
                wd_t.append(wd)
                # boot scan (y'[-1] = scan of u_rev[0..126])
                zp = cst.tile([128, 127], F32, tag=f"zp{k}")
                nc.vector.tensor_tensor_scan(
                    zp[:],
                    r_t[:, k:k + 1].broadcast_to([128, 127]),
                    xt[(0, k)][:, 1:128].bitcast(F32),
                    0.0, AL.mult, AL.add,
                )
                ylast[k] = zp[:, 126:127]

            # ---- chunk-0 gate, k-major (one PSUM chain per e-tile j), so each
            # arriving wd[k] is consumed immediately
            g0_t = []
            with tc.tile_pool(name="ps0", bufs=1, space="PSUM") as ps0:
                pg0 = []
                for j in range(KD):
                    pg0_j = ps0.tile([128, CH], F32, tag=f"pg0{j}")
                    pg0.append(pg0_j)
                for k in range(KD):
                    for j in range(KD):
                        nc.tensor.matmul(
                            pg0[j][:],
                            wd_t[k][:, j * 128:(j + 1) * 128],
                            xt[(0, k)][:, 1:513],
                            start=(k == 0), stop=(k == KD - 1),
                        )
                for j in range(KD):
                    g = gp.tile([128, CH], F32, tag=f"g{j}")
                    nc.scalar.activation(g[:], pg0[j][:], ACTF.Sigmoid,
                                         bias=bd_t[:, j:j + 1])
                    g0_t.append(g)

            # ---- x chunk 1 prefetch, then Wo
            for k in range(KD):
                t = xp.tile([128, 640], F32R, tag=f"x{k}")
                nc.sync.dma_start(t[:], XT[k * 128:(k + 1) * 128, CH:CH + 640])
                xt[(1, k)] = t
            for k in range(KD):
                wo = wgt.tile([128, D], F32R, tag=f"wo{k}")
                nc.sync.dma_start(wo[:], WOT[k * 128:(k + 1) * 128, :])
                wo_t.append(wo)

            # ---- main chunk loop
            with (
                tc.tile_pool(name="psg", bufs=3, space="PSUM") as psg,
                tc.tile_pool(name="pso", bufs=3, space="PSUM") as pso,
            ):
              for c in range(NCH):
                if 1 <= c and c + 1 < NCH:
                    for k in range(KD):
                        t = xp.tile([128, 640], F32R, tag=f"x{k}")
                        nc.sync.dma_start(
                            t[:], XT[k * 128:(k + 1) * 128, (c + 1) * CH:(c + 1) * CH + 640])
                        xt[(c + 1, k)] = t

                # gate matmuls + sigmoid (PE/ACT; independent of scan chain)
                if c == 0:
                    g_t = g0_t
                else:
                    g_t = []
                    for j in range(KD):
                        pg = psg.tile([128, CH], F32, tag="pg")
                        for k in range(KD):
                            nc.tensor.matmul(
                                pg[:],
                                wd_t[k][:, j * 128:(j + 1) * 128],
                                xt[(c, k)][:, 1:513],
                                start=(k == 0), stop=(k == KD - 1),
                            )
                        g = gp.tile([128, CH], F32, tag=f"g{j}")
                        nc.scalar.activation(g[:], pg[:], ACTF.Sigmoid, bias=bd_t[:, j:j + 1])
                        g_t.append(g)

                # scan chain + gating + residual (DVE)
                v_t = []
                for k in range(KD):
                    xa = xt[(c, k)]
                    d1 = d1p.tile([128, CH], F32, tag="d")
                    nc.vector.scalar_tensor_tensor(
                        d1[:], xa[:, 0:512].bitcast(F32), nr_t[:, k:k + 1],
                        xa[:, 128:640].bitcast(F32), AL.mult, AL.add)
                    y = yp.tile([128, CH], F32, tag=f"y{k}")
                    nc.vector.tensor_tensor_scan(
                        y[:], r_t[:, k:k + 1].broadcast_to([128, CH]), d1[:],
                        ylast[k], AL.mult, AL.add)
                    ylast[k] = y[:, CH - 1:CH]
                    w = wp.tile([128, CH], F32, tag="w")
                    nc.vector.scalar_tensor_tensor(
                        w[:], y[:], sc_t[:, k:k + 1], g_t[k][:], AL.mult, AL.mult)
                    v = vp.tile([128, CH], F32R, tag=f"v{k}")
                    nc.vector.tensor_tensor(
                        v[:], w[:], xa[:, 1:513].bitcast(F32), AL.add)
                    v_t.append(v)

                # output projection + bias + store
                for j in range(KD):
                    po = pso.tile([128, CH], F32, tag="po")
                    for k in range(KD):
                        nc.tensor.matmul(
                            po[:],
                            wo_t[k][:, j * 128:(j + 1) * 128],
                            v_t[k][:],
                            start=(k == 0), stop=(k == KD - 1),
                        )
                    o = op.tile([128, CH], F32, tag=f"o{j}")
                    nc.scalar.activation(o[:], po[:], ACTF.Identity, bias=bo_t[:, j:j + 1])
                    nc.sync.dma_start(
                        OUT[j * 128:(j + 1) * 128, c * CH:(c + 1) * CH], o[:])

    nc.compile()
    _CACHE["nc"] = nc
    return nc


def _pack_vec(v):
    # [D] -> [128, KD] with vec[p, k] = v[k*128+p]
    return np.ascontiguousarray(v.astype(np.float32).reshape(KD, 128).T)


def kernel(x, raw_gamma, raw_beta, C, Wd, bd, Wo, bo):
    x = np.asarray(x)
    in_dtype = x.dtype
    # host precompute of per-channel scalars (f64 -> f32)
    rg = np.asarray(raw_gamma, np.float64)
    rb = np.asarray(raw_beta, np.float64)
    C64 = np.asarray(C, np.float64)
    gamma = np.log1p(np.exp(-np.abs(rg))) + np.maximum(rg, 0.0) + 1e-4  # softplus
    r = np.exp(-gamma)
    decay = np.exp(-np.arange(128)[None, :] * gamma[:, None])
    ker = C64[:, None] * decay
    norm = np.sqrt((ker * ker).sum(-1))
    A = C64 / (norm + 1e-6)
    scale = np.tanh(rb) * A
    r128 = r ** 128

    nc = _build()

    xf = np.asarray(x, np.float32).reshape(B * T, D)
    wdt = np.ascontiguousarray(np.asarray(Wd, np.float32).T)
    wot = np.ascontiguousarray(np.asarray(Wo, np.float32).T)
    cv = np.concatenate([
        _pack_vec(r), _pack_vec(-r128), _pack_vec(scale),
        _pack_vec(np.asarray(bd, np.float64)), _pack_vec(np.asarray(bo, np.float64)),
    ], axis=1)

    in_maps = []
    for c in range(NCORE):
        t0 = c * TC
        b = t0 // T
        lo = max(b * T, t0 - 128)
        # x_rev: col 0 = zero, cols 1..TC = chunk reversed, then halo reversed, zero pad
        xr = np.zeros((XW, D), np.float32)
        xr[1:TC + 1] = xf[t0:t0 + TC][::-1]
        nh = t0 - lo
        if nh:
            xr[TC + 1:TC + 1 + nh] = xf[lo:t0][::-1]
        in_maps.append({
            "xt": np.ascontiguousarray(xr.T),
            "wdt": wdt, "wot": wot, "cv": cv,
        })

    _CACHE["last_in_maps"] = in_maps
    res = bass_utils.run_bass_kernel_spmd(nc, in_maps, core_ids=list(range(NCORE)))
    _CACHE["last_result"] = res

    out = np.empty((B * T, D), np.float32)
    for c in range(NCORE):
        o = res.results[c]["out"]                      # [D, TC] reversed time
        out[c * TC:(c + 1) * TC] = o[:, ::-1].T
    return out.reshape(B, T, D).astype(in_dtype, copy=False)


# revision 15
# speedup vs baseline: 1.1885x; 1.1316x over previous
# Trainium2 Bass kernel for nn_SSM: gated exponential-FIR depthwise conv + projections.
#
#   dynamic_scale = sigmoid(x @ Wd^T + bd)
#   u  = x * tanh(raw_beta)
#   y  = causal depthwise FIR conv (K=128) of u, kernel A[d]*r[d]^k (L2-normalized),
#        cross-correlated => lag-j tap is A*r^(127-j)
#   out = (y * dynamic_scale + x) @ Wo^T + bo
#
# Data-parallel over tokens (B*T = 16384 -> 2048/core, 8 cores), halo of 127 past
# tokens per core. On-chip everything runs in TIME-REVERSED transposed layout
# [channel, tau'] so the backward FIR becomes a forward first-order scan:
#     y'[tau'] = r*y'[tau'-1] + (u[tau'+127] - r^128 * u[tau'-1])   (hw tensor_tensor_scan)
# with the per-channel scale (tanh(beta)*A) folded into the gating multiply.
# Matmuls run in fp32r (fp32 with 11-bit mantissa) at full PE rate, N=512.
import numpy as np
import concourse.bacc as bacc
import concourse.tile as tile
import concourse.mybir as mybir
from concourse import bass_utils

F32 = mybir.dt.float32
F32R = mybir.dt.float32r
AL = mybir.AluOpType
ACTF = mybir.ActivationFunctionType

B, T, D = 4, 4096, 1024
NCORE = 8
TC = (B * T) // NCORE          # tokens per core (2048)
CH = 512                       # time chunk
NCH = TC // CH                 # 4 chunks
KD = D // 128                  # 8 channel tiles
XW = TC + 129                  # x columns: col j = u_rev[j-1], col 0 is zero pad

_CACHE = {}


def _build():
    if "nc" in _CACHE:
        return _CACHE["nc"]
    nc = bacc.Bacc("TRN2", target_bir_lowering=False, debug=False, num_devices=NCORE)

    XT = nc.dram_tensor("xt", [D, XW], F32R, kind="ExternalInput")
    WDT = nc.dram_tensor("wdt", [D, D], F32R, kind="ExternalInput")
    WOT = nc.dram_tensor("wot", [D, D], F32R, kind="ExternalInput")
    # per-channel vectors packed [128, 5*KD]: KD cols each of
    # r, -r^128, tanh(beta)*A, bd, bo;  vec[p, k] = v[k*128 + p]
    CV = nc.dram_tensor("cv", [128, 5 * KD], F32, kind="ExternalInput")
    OUT = nc.dram_tensor("out", [D, TC], F32, kind="ExternalOutput")

    with tile.TileContext(nc) as tc:
        with (
            tc.tile_pool(name="wgt", bufs=1) as wgt,
            tc.tile_pool(name="cst", bufs=1) as cst,
            tc.tile_pool(name="xp", bufs=2) as xp,
            tc.tile_pool(name="d1p", bufs=2) as d1p,
            tc.tile_pool(name="yp", bufs=2) as yp,
            tc.tile_pool(name="wp", bufs=2) as wp,
            tc.tile_pool(name="vp", bufs=1) as vp,
            tc.tile_pool(name="gp", bufs=1) as gp,
            tc.tile_pool(name="op", bufs=1) as op,
        ):
            # ---- consts + x chunk 0 first (unblocks DVE boot + first chunk fast)
            cv_t = cst.tile([128, 5 * KD], F32, tag="cv")
            nc.sync.dma_start(cv_t[:], CV[:])
            r_t = cv_t[:, 0 * KD:1 * KD]
            nr_t = cv_t[:, 1 * KD:2 * KD]
            sc_t = cv_t[:, 2 * KD:3 * KD]
            bd_t = cv_t[:, 3 * KD:4 * KD]
            bo_t = cv_t[:, 4 * KD:5 * KD]

            # ---- interleaved x0/Wd DMA pairs + boot scans, so the first gate
            # matmuls can start as soon as x0[0]+wd[0] land.
            xt = {}
            wd_t, wo_t = [], []
            ylast = [None] * KD
            for k in range(KD):
                t = xp.tile([128, 640], F32R, tag=f"x{k}")
                nc.sync.dma_start(t[:], XT[k * 128:(k + 1) * 128, 0:640])
                xt[(0, k)] = t
                wd = wgt.tile([128, D], F32R, tag=f"wd{k}")
                nc.scalar.dma_start(wd[:], WDT[k * 128:(k + 1) * 128, :])
                wd_t.append(wd)
                # boot scan (y'[-1] = scan of u_rev[0..126])
                zp = cst.tile([128, 127], F32, tag=f"zp{k}")
                nc.vector.tensor_tensor_scan(
                    zp[:],
                    r_t[:, k:k + 1].broadcast_to([128, 127]),
                    xt[(0, k)][:, 1:128].bitcast(F32),
                    0.0, AL.mult, AL.add,
                )
                ylast[k] = zp[:, 126:127]

            # ---- chunk-0 gate, k-major (one PSUM chain per e-tile j), so each
            # arriving wd[k] is consumed immediately
            g0_t = []
            with tc.tile_pool(name="ps0", bufs=1, space="PSUM") as ps0:
                pg0 = []
                for j in range(KD):
                    pg0_j = ps0.tile([128, CH], F32, tag=f"pg0{j}")
                    pg0.append(pg0_j)
                for k in range(KD):
                    for j in range(KD):
                        nc.tensor.matmul(
                            pg0[j][:],
                            wd_t[k][:, j * 128:(j + 1) * 128],
                            xt[(0, k)][:, 1:513],
                            start=(k == 0), stop=(k == KD - 1),
                        )
                for j in range(KD):
                    g = gp.tile([128, CH], F32, tag=f"g{j}")
                    nc.scalar.activation(g[:], pg0[j][:], ACTF.Sigmoid,
                                         bias=bd_t[:, j:j + 1])
                    g0_t.append(g)

            # ---- x chunk 1 prefetch, then Wo
            for k in range(KD):
                t = xp.tile([128, 640], F32R, tag=f"x{k}")
                nc.sync.dma_start(t[:], XT[k * 128:(k + 1) * 128, CH:CH + 640])
                xt[(1, k)] = t
            for k in range(KD):
                wo = wgt.tile([128, D], F32R, tag=f"wo{k}")
                nc.sync.dma_start(wo[:], WOT[k * 128:(k + 1) * 128, :])
                wo_t.append(wo)

            # ---- main chunk loop
            with (
                tc.tile_pool(name="psg", bufs=3, space="PSUM") as psg,
                tc.tile_pool(name="pso", bufs=3, space="PSUM") as pso,
            ):
              for c in range(NCH):
                if 1 <= c and c + 1 < NCH:
                    for k in range(KD):
                        t = xp.tile([128, 640], F32R, tag=f"x{k}")
                        nc.sync.dma_start(
                            t[:], XT[k * 128:(k + 1) * 128, (c + 1) * CH:(c + 1) * CH + 640])
                        xt[(c + 1, k)] = t

                # gate matmuls + sigmoid (PE/ACT; independent of scan chain)
                if c == 0:
                    g_t = g0_t
                else:
                    g_t = []
                    for j in range(KD):
                        pg = psg.tile([128, CH], F32, tag="pg")
                        for k in range(KD):
                            nc.tensor.matmul(
                                pg[:],
                                wd_t[k][:, j * 128:(j + 1) * 128],
                                xt[(c, k)][:, 1:513],
                                start=(k == 0), stop=(k == KD - 1),
                            )
                        g = gp.tile([128, CH], F32, tag=f"g{j}")
                        nc.scalar.activation(g[:], pg[:], ACTF.Sigmoid, bias=bd_t[:, j:j + 1])
                        g_t.append(g)

                # scan chain + gating + residual (DVE)
                v_t = []
                for k in range(KD):
                    xa = xt[(c, k)]
                    d1 = d1p.tile([128, CH], F32, tag="d")
                    nc.vector.scalar_tensor_tensor(
                        d1[:], xa[:, 0:512].bitcast(F32), nr_t[:, k:k + 1],
                        xa[:, 128:640].bitcast(F32), AL.mult, AL.add)
                    y = yp.tile([128, CH], F32, tag=f"y{k}")
                    nc.vector.tensor_tensor_scan(
                        y[:], r_t[:, k:k + 1].broadcast_to([128, CH]), d1[:],
                        ylast[k], AL.mult, AL.add)
                    ylast[k] = y[:, CH - 1:CH]
                    w = wp.tile([128, CH], F32, tag="w")
                    nc.vector.scalar_tensor_tensor(
                        w[:], y[:], sc_t[:, k:k + 1], g_t[k][:], AL.mult, AL.mult)
                    v = vp.tile([128, CH], F32R, tag=f"v{k}")
                    nc.vector.tensor_tensor(
                        v[:], w[:], xa[:, 1:513].bitcast(F32), AL.add)
                    v_t.append(v)

                # output projection + bias + store
                for j in range(KD):
                    po = pso.tile([128, CH], F32, tag="po")
                    for k in range(KD):
                        nc.tensor.matmul(
                            po[:],
                            wo_t[k][:, j * 128:(j + 1) * 128],
                            v_t[k][:],
                            start=(k == 0), stop=(k == KD - 1),
                        )
                    o = op.tile([128, CH], F32, tag=f"o{j}")
                    nc.scalar.activation(o[:], po[:], ACTF.Identity, bias=bo_t[:, j:j + 1])
                    nc.sync.dma_start(
                        OUT[j * 128:(j + 1) * 128, c * CH:(c + 1) * CH], o[:])

    nc.compile()
    _CACHE["nc"] = nc
    return nc


def _pack_vec(v):
    # [D] -> [128, KD] with vec[p, k] = v[k*128+p]
    return np.ascontiguousarray(v.astype(np.float32).reshape(KD, 128).T)


def kernel(x, raw_gamma, raw_beta, C, Wd, bd, Wo, bo):
    x = np.asarray(x)
    in_dtype = x.dtype
    # host precompute of per-channel scalars (f64 -> f32)
    rg = np.asarray(raw_gamma, np.float64)
    rb = np.asarray(raw_beta, np.float64)
    C64 = np.asarray(C, np.float64)
    gamma = np.log1p(np.exp(-np.abs(rg))) + np.maximum(rg, 0.0) + 1e-4  # softplus
    r = np.exp(-gamma)
    decay = np.exp(-np.arange(128)[None, :] * gamma[:, None])
    ker = C64[:, None] * decay
    norm = np.sqrt((ker * ker).sum(-1))
    A = C64 / (norm + 1e-6)
    scale = np.tanh(rb) * A
    r128 = r ** 128

    nc = _build()

    xf = np.asarray(x, np.float32).reshape(B * T, D)
    wdt = np.ascontiguousarray(np.asarray(Wd, np.float32).T)
    wot = np.ascontiguousarray(np.asarray(Wo, np.float32).T)
    cv = np.concatenate([
        _pack_vec(r), _pack_vec(-r128), _pack_vec(scale),
        _pack_vec(np.asarray(bd, np.float64)), _pack_vec(np.asarray(bo, np.float64)),
    ], axis=1)

    in_maps = []
    for c in range(NCORE):
        t0 = c * TC
        b = t0 // T
        lo = max(b * T, t0 - 128)
        # x_rev: col 0 = zero, cols 1..TC = chunk reversed, then halo reversed, zero pad
        xr = np.zeros((XW, D), np.float32)
        xr[1:TC + 1] = xf[t0:t0 + TC][::-1]
        nh = t0 - lo
        if nh:
            xr[TC + 1:TC + 1 + nh] = xf[lo:t0][::-1]
        in_maps.append({
            "xt": np.ascontiguousarray(xr.T),
            "wdt": wdt, "wot": wot, "cv": cv,
        })

    _CACHE["last_in_maps"] = in_maps
    res = bass_utils.run_bass_kernel_spmd(nc, in_maps, core_ids=list(range(NCORE)))
    _CACHE["last_result"] = res

    out = np.empty((B * T, D), np.float32)
    for c in range(NCORE):
        o = res.results[c]["out"]                      # [D, TC] reversed time
        out[c * TC:(c + 1) * TC] = o[:, ::-1].T
    return out.reshape(B, T, D).astype(in_dtype, copy=False)


# revision 16
# speedup vs baseline: 1.3034x; 1.0967x over previous
# Trainium2 Bass kernel for nn_SSM: gated exponential-FIR depthwise conv + projections.
#
#   dynamic_scale = sigmoid(x @ Wd^T + bd)
#   u  = x * tanh(raw_beta)
#   y  = causal depthwise FIR conv (K=128) of u, kernel A[d]*r[d]^k (L2-normalized),
#        cross-correlated => lag-j tap is A*r^(127-j)
#   out = (y * dynamic_scale + x) @ Wo^T + bo
#
# Data-parallel over tokens (B*T = 16384 -> 2048/core, 8 cores), halo of 127 past
# tokens per core. On-chip everything runs in TIME-REVERSED transposed layout
# [channel, tau'] so the backward FIR becomes a forward first-order scan:
#     y'[tau'] = r*y'[tau'-1] + (u[tau'+127] - r^128 * u[tau'-1])   (hw tensor_tensor_scan)
# with the per-channel scale (tanh(beta)*A) folded into the gating multiply.
# Matmuls run in fp32r (fp32 with 11-bit mantissa) at full PE rate, N=512.
import numpy as np
import concourse.bacc as bacc
import concourse.tile as tile
import concourse.mybir as mybir
from concourse import bass_utils

F32 = mybir.dt.float32
F32R = mybir.dt.float32r
AL = mybir.AluOpType
ACTF = mybir.ActivationFunctionType

B, T, D = 4, 4096, 1024
NCORE = 8
TC = (B * T) // NCORE          # tokens per core (2048)
CH = 512                       # time chunk
NCH = TC // CH                 # 4 chunks
KD = D // 128                  # 8 channel tiles
XW = TC + 129                  # x columns: col j = u_rev[j-1], col 0 is zero pad

_CACHE = {}


def _build():
    if "nc" in _CACHE:
        return _CACHE["nc"]
    nc = bacc.Bacc("TRN2", target_bir_lowering=False, debug=False, num_devices=NCORE)

    XT = nc.dram_tensor("xt", [D, XW], F32R, kind="ExternalInput")
    WDT = nc.dram_tensor("wdt", [D, D], F32R, kind="ExternalInput")
    WOT = nc.dram_tensor("wot", [D, D], F32R, kind="ExternalInput")
    # per-channel vectors packed [128, 5*KD]: KD cols each of
    # r, -r^128, tanh(beta)*A, bd, bo;  vec[p, k] = v[k*128 + p]
    CV = nc.dram_tensor("cv", [128, 5 * KD], F32, kind="ExternalInput")
    OUT = nc.dram_tensor("out", [D, TC], F32, kind="ExternalOutput")

    with tile.TileContext(nc) as tc:
        with (
            tc.tile_pool(name="wgt", bufs=1) as wgt,
            tc.tile_pool(name="cst", bufs=1) as cst,
            tc.tile_pool(name="xp", bufs=2) as xp,
            tc.tile_pool(name="d1p", bufs=2) as d1p,
            tc.tile_pool(name="yp", bufs=2) as yp,
            tc.tile_pool(name="wp", bufs=2) as wp,
            tc.tile_pool(name="vp", bufs=2) as vp,
            tc.tile_pool(name="gp", bufs=1) as gp,
            tc.tile_pool(name="op", bufs=3) as op,
        ):
            # ---- consts + x chunk 0 first (unblocks DVE boot + first chunk fast)
            cv_t = cst.tile([128, 5 * KD], F32, tag="cv")
            nc.sync.dma_start(cv_t[:], CV[:])
            r_t = cv_t[:, 0 * KD:1 * KD]
            nr_t = cv_t[:, 1 * KD:2 * KD]
            sc_t = cv_t[:, 2 * KD:3 * KD]
            bd_t = cv_t[:, 3 * KD:4 * KD]
            bo_t = cv_t[:, 4 * KD:5 * KD]

            # ---- interleaved x0/Wd DMA pairs + boot scans, so the first gate
            # matmuls can start as soon as x0[0]+wd[0] land.
            xt = {}
            wd_t, wo_t = [], []
            ylast = [None] * KD
            for k in range(KD):
                t = xp.tile([128, 640], F32R, tag=f"x{k}")
                nc.sync.dma_start(t[:], XT[k * 128:(k + 1) * 128, 0:640])
                xt[(0, k)] = t
                wd = wgt.tile([128, D], F32R, tag=f"wd{k}")
                nc.scalar.dma_start(wd[:], WDT[k * 128:(k + 1) * 128, :])
                wd_t.append(wd)
                # boot scan (y'[-1] = scan of u_rev[0..126])
                zp = cst.tile([128, 127], F32, tag=f"zp{k}")
                nc.vector.tensor_tensor_scan(
                    zp[:],
                    r_t[:, k:k + 1].broadcast_to([128, 127]),
                    xt[(0, k)][:, 1:128].bitcast(F32),
                    0.0, AL.mult, AL.add,
                )
                ylast[k] = zp[:, 126:127]

            # ---- chunk-0 gate, k-major (one PSUM chain per e-tile j), so each
            # arriving wd[k] is consumed immediately
            g0_t = []
            with tc.tile_pool(name="ps0", bufs=1, space="PSUM") as ps0:
                pg0 = []
                for j in range(KD):
                    pg0_j = ps0.tile([128, CH], F32, tag=f"pg0{j}")
                    pg0.append(pg0_j)
                for k in range(KD):
                    for j in range(KD):
                        nc.tensor.matmul(
                            pg0[j][:],
                            wd_t[k][:, j * 128:(j + 1) * 128],
                            xt[(0, k)][:, 1:513],
                            start=(k == 0), stop=(k == KD - 1),
                        )
                for j in range(KD):
                    g = gp.tile([128, CH], F32, tag=f"g{j}")
                    nc.scalar.activation(g[:], pg0[j][:], ACTF.Sigmoid,
                                         bias=bd_t[:, j:j + 1])
                    g0_t.append(g)

            # ---- x chunk 1 prefetch, then Wo
            for k in range(KD):
                t = xp.tile([128, 640], F32R, tag=f"x{k}")
                nc.sync.dma_start(t[:], XT[k * 128:(k + 1) * 128, CH:CH + 640])
                xt[(1, k)] = t
            for k in range(KD):
                wo = wgt.tile([128, D], F32R, tag=f"wo{k}")
                nc.sync.dma_start(wo[:], WOT[k * 128:(k + 1) * 128, :])
                wo_t.append(wo)

            # ---- main chunk loop
            with (
                tc.tile_pool(name="psg", bufs=3, space="PSUM") as psg,
                tc.tile_pool(name="pso", bufs=3, space="PSUM") as pso,
            ):
              def emit_out(c, v_t):
                for j in range(KD):
                    po = pso.tile([128, CH], F32, tag="po")
                    for k in range(KD):
                        nc.tensor.matmul(
                            po[:],
                            wo_t[k][:, j * 128:(j + 1) * 128],
                            v_t[k][:],
                            start=(k == 0), stop=(k == KD - 1),
                        )
                    o = op.tile([128, CH], F32, tag="o")
                    nc.scalar.activation(o[:], po[:], ACTF.Identity, bias=bo_t[:, j:j + 1])
                    nc.sync.dma_start(
                        OUT[j * 128:(j + 1) * 128, c * CH:(c + 1) * CH], o[:])

              prev_v = None
              for c in range(NCH):
                if 1 <= c and c + 1 < NCH:
                    for k in range(KD):
                        t = xp.tile([128, 640], F32R, tag=f"x{k}")
                        nc.sync.dma_start(
                            t[:], XT[k * 128:(k + 1) * 128, (c + 1) * CH:(c + 1) * CH + 640])
                        xt[(c + 1, k)] = t

                # gate matmuls + sigmoid (PE/ACT; independent of scan chain)
                if c == 0:
                    g_t = g0_t
                else:
                    g_t = []
                    for j in range(KD):
                        pg = psg.tile([128, CH], F32, tag="pg")
                        for k in range(KD):
                            nc.tensor.matmul(
                                pg[:],
                                wd_t[k][:, j * 128:(j + 1) * 128],
                                xt[(c, k)][:, 1:513],
                                start=(k == 0), stop=(k == KD - 1),
                            )
                        g = gp.tile([128, CH], F32, tag=f"g{j}")
                        nc.scalar.activation(g[:], pg[:], ACTF.Sigmoid, bias=bd_t[:, j:j + 1])
                        g_t.append(g)

                # scan chain + gating + residual (DVE)
                v_t = []
                for k in range(KD):
                    xa = xt[(c, k)]
                    d1 = d1p.tile([128, CH], F32, tag="d")
                    nc.vector.scalar_tensor_tensor(
                        d1[:], xa[:, 0:512].bitcast(F32), nr_t[:, k:k + 1],
                        xa[:, 128:640].bitcast(F32), AL.mult, AL.add)
                    y = yp.tile([128, CH], F32, tag=f"y{k}")
                    nc.vector.tensor_tensor_scan(
                        y[:], r_t[:, k:k + 1].broadcast_to([128, CH]), d1[:],
                        ylast[k], AL.mult, AL.add)
                    ylast[k] = y[:, CH - 1:CH]
                    w = wp.tile([128, CH], F32, tag="w")
                    nc.vector.scalar_tensor_tensor(
                        w[:], y[:], sc_t[:, k:k + 1], g_t[k][:], AL.mult, AL.mult)
                    v = vp.tile([128, CH], F32R, tag=f"v{k}")
                    nc.vector.tensor_tensor(
                        v[:], w[:], xa[:, 1:513].bitcast(F32), AL.add)
                    v_t.append(v)

                # previous chunk's output projection (keeps PE stream busy while
                # this chunk's DVE chain runs)
                if prev_v is not None:
                    emit_out(c - 1, prev_v)
                prev_v = v_t
              emit_out(NCH - 1, prev_v)

    nc.compile()
    _CACHE["nc"] = nc
    return nc


def _pack_vec(v):
    # [D] -> [128, KD] with vec[p, k] = v[k*128+p]
    return np.ascontiguousarray(v.astype(np.float32).reshape(KD, 128).T)


def kernel(x, raw_gamma, raw_beta, C, Wd, bd, Wo, bo):
    x = np.asarray(x)
    in_dtype = x.dtype
    # host precompute of per-channel scalars (f64 -> f32)
    rg = np.asarray(raw_gamma, np.float64)
    rb = np.asarray(raw_beta, np.float64)
    C64 = np.asarray(C, np.float64)
    gamma = np.log1p(np.exp(-np.abs(rg))) + np.maximum(rg, 0.0) + 1e-4  # softplus
    r = np.exp(-gamma)
    decay = np.exp(-np.arange(128)[None, :] * gamma[:, None])
    ker = C64[:, None] * decay
    norm = np.sqrt((ker * ker).sum(-1))
    A = C64 / (norm + 1e-6)
    scale = np.tanh(rb) * A
    r128 = r ** 128

    nc = _build()

    xf = np.asarray(x, np.float32).reshape(B * T, D)
    wdt = np.ascontiguousarray(np.asarray(Wd, np.float32).T)
    wot = np.ascontiguousarray(np.asarray(Wo, np.float32).T)
    cv = np.concatenate([
        _pack_vec(r), _pack_vec(-r128), _pack_vec(scale),
        _pack_vec(np.asarray(bd, np.float64)), _pack_vec(np.asarray(bo, np.float64)),
    ], axis=1)

    in_maps = []
    for c in range(NCORE):
        t0 = c * TC
        b = t0 // T
        lo = max(b * T, t0 - 128)
        # x_rev: col 0 = zero, cols 1..TC = chunk reversed, then halo reversed, zero pad
        xr = np.zeros((XW, D), np.float32)
        xr[1:TC + 1] = xf[t0:t0 + TC][::-1]
        nh = t0 - lo
        if nh:
            xr[TC + 1:TC + 1 + nh] = xf[lo:t0][::-1]
        in_maps.append({
            "xt": np.ascontiguousarray(xr.T),
            "wdt": wdt, "wot": wot, "cv": cv,
        })

    _CACHE["last_in_maps"] = in_maps
    res = bass_utils.run_bass_kernel_spmd(nc, in_maps, core_ids=list(range(NCORE)))
    _CACHE["last_result"] = res

    out = np.empty((B * T, D), np.float32)
    for c in range(NCORE):
        o = res.results[c]["out"]                      # [D, TC] reversed time
        out[c * TC:(c + 1) * TC] = o[:, ::-1].T
    return out.reshape(B, T, D).astype(in_dtype, copy=False)
